# revision 25
# baseline (speedup 1.0000x reference)
"""DynamicUpsamplingFilter kernel for Trainium2 (Bass/Tile), 8 NeuronCores.

out[b, c*16+r, h, w] = sum_{di,dj} x_pad[b, c, h+di, w+dj] * filters[b, di*5+dj, r, h, w]

Sharding: purely data parallel - one batch element per NeuronCore (B=8).

Per-core dataflow (PE-centric; the per-pixel [3x25]@[25x16] contraction runs
directly on the tensor engine):
  * Image rows are grouped in chunks of PG=4 rows (NG=45 per core). Partition
    p = 32*pix + f holds tap f (of 25) for row-in-group pix (of 4); partitions
    32*pix+25..31 are dead (host ships zeros there; the matching stationary
    weight rows also stay zero, so those lanes contribute nothing).
  * Host prepacks 3 groups per DMA ("triples"). Filters go as fp8 E3M4 -
    filter values are uniform [0,1) so 4 mantissa bits keep the output L2
    error at ~1.24e-2, under the 2e-2 gate - and x windows go as fp16:
      ftd[t, p, (gsub, w, r)] = filters[f, r, 4(3t+gsub)+pix, w],  p = 32*pix+f
      xwd[t, p, (gsub, w, c)] = x_pad[c, 4(3t+gsub)+pix+di-2, w+dj-2]
  * DVE scatters xwd into per-group block-diagonal stationary weights
      w5b[32*pix+f, w, 3*pix+c] = xwd[...]   (all other slots stay zero)
    so for every (group, w) the [128, 16] stationary W holds the 4 pixels'
    25-tap patches on its block diagonal (cols 12..15 zero).
  * PE: ONE matmul per (group, w): out[16, 16] = W.T @ ftd[:, w, :] computes
    all 48 outputs (3 channels x 16 r) for 4 pixels at column w in a single
    instruction; fp8 moving operand, fp16 stationary, fp32 psum accumulate.
    Outputs land in psum col-strips via tile_position (0, 32j).
  * ACT drains psum -> SBUF fp16 into a per-triple staging tile; 4 strip
    stores per triple (partition base 32j, 12 rows each) write only the
    useful rows. The host reassembles the fp32 output (pure layout work,
    no arithmetic).
Measured: TimelineSim ~139.8 us per core (baseline was 413.6 us); verified on
8x TRN2 NeuronCores with L2 rel err 1.24e-2 vs the fp32 reference.
"""

import numpy as np

import concourse.bass as bass
import concourse.bacc as bacc
import concourse.mybir as mybir
from concourse.tile import TileContext
from concourse.bass_utils import run_bass_kernel_spmd

B, C, H, W = 8, 3, 180, 320
NF, R = 25, 16
K, PAD = 5, 2
PG = 4  # rows per group
NG = H // PG  # 45 groups
NT = NG // 3  # 15 triples
WR = W * R
WC = W * C
NFT = 2  # ft triple buffers
NXW = 2  # xw triple buffers
NST = 3  # store staging buffers
NW5 = 3  # block-diag weight buffers

DT = mybir.dt.float16
DT8 = mybir.dt.float8e3
F32 = mybir.dt.float32
I32 = mybir.dt.int32

_CACHED = {}


def _build_nc():
    nc = bacc.Bacc("TRN2", target_bir_lowering=False, debug=False, num_devices=8)
    ftd = nc.dram_tensor("ftd", [NT, 128, 3 * WR], DT8, kind="ExternalInput")
    xwd = nc.dram_tensor("xwd", [NT, 122, 3 * WC], DT, kind="ExternalInput")
    od = nc.dram_tensor("od", [NT, 4, 12, 3 * 1280], DT, kind="ExternalOutput")

    with TileContext(nc) as tc:
        with (
            tc.tile_pool(name="p", bufs=1) as pool,
            tc.tile_pool(name="ps", bufs=1, space="PSUM") as psp,
        ):
            w5bufs = [
                pool.tile([128, W, 16], DT, tag=f"w5{i}", name=f"w5{i}")
                for i in range(NW5)
            ]
            engs = [nc.vector, nc.gpsimd]
            for i, t in enumerate(w5bufs):
                engs[i % 2].memset(t[:].bitcast(I32), 0)
            ft_tiles = [
                pool.tile([128, 3, W, R], DT8, tag=f"ft{i}", name=f"ftt{i}")
                for i in range(NFT)
            ]
            xw_tiles = [
                pool.tile([128, 3, W, C], DT, tag=f"xw{i}", name=f"xwt{i}")
                for i in range(NXW)
            ]
            st_tiles = [
                pool.tile([128, 3, 1280], DT, tag=f"st{i}", name=f"stt{i}")
                for i in range(NST)
            ]
            for g in range(NG):
                t, gsub = g // 3, g % 3
                w5b = w5bufs[g % NW5]
                ftt = ft_tiles[t % NFT]
                xwt = xw_tiles[t % NXW]
                st = st_tiles[t % NST]
                if gsub == 0:
                    nc.sync.dma_start(
                        out=bass.AP(
                            xwt[:].tensor, 0, [[3 * WC, 122], [1, 3 * WC]]
                        ),
                        in_=xwd[t],
                    )
                    nc.sync.dma_start(
                        out=bass.AP(
                            ftt[:].tensor, 0, [[3 * WR, 128], [1, 3 * WR]]
                        ),
                        in_=ftd[t],
                    )
                # scatter the 4-pixel patch blocks into the block-diag weights
                for pix in range(PG):
                    nc.vector.tensor_copy(
                        out=w5b[32 * pix : 32 * pix + NF, :, 3 * pix : 3 * pix + 3],
                        in_=xwt[32 * pix : 32 * pix + NF, gsub],
                    )
                pa = psp.tile([128, 1024], F32, tag="pa", bufs=2, name="pa")
                pc = psp.tile([128, 512], F32, tag="pc", bufs=2, name="pc")
                for w in range(W):
                    if w < 256:
                        j, blk, s = (w % 128) // 32, w // 128, w % 32
                        out = pa[
                            32 * j : 32 * j + 16,
                            512 * blk + 16 * s : 512 * blk + 16 * s + 16,
                        ]
                    else:
                        j, s = (w - 256) // 16, (w - 256) % 16
                        out = pc[32 * j : 32 * j + 16, 16 * s : 16 * s + 16]
                    nc.tensor.matmul(
                        out,
                        w5b[:, w, :],
                        ftt[:, gsub, w, :],
                        start=True,
                        stop=True,
                        tile_position=(0, 32 * j),
                    )
                nc.scalar.copy(out=st[:, gsub, :1024], in_=pa)
                nc.scalar.copy(out=st[:, gsub, 1024:1280], in_=pc[:, :256])
                if gsub == 2:
                    for j in range(4):
                        # last triple: alternate issue queues so the final
                        # stores' issue latency overlaps
                        eng = nc.sync if (t == NT - 1 and j % 2) else nc.scalar
                        eng.dma_start(
                            out=od[t, j],
                            in_=st[32 * j : 32 * j + 12],
                        )

    nc.compile()
    return nc


def _get_nc():
    if "nc" not in _CACHED:
        _CACHED["nc"] = _build_nc()
    return _CACHED["nc"]


def _prep_maps(x, filters):
    x = np.asarray(x)
    filters = np.asarray(filters)
    # ftd[b, t, 32*pix+f, (gsub, w, r)] = filters[b, f, r, 4*(3t+gsub)+pix, w]
    ftq = (
        filters.astype(mybir.dt.np(DT8))
        .transpose(0, 3, 1, 4, 2)  # [B, H, 25, W, 16]
        .reshape(B, NT, 3, PG, NF, WR)
    )
    ftp = np.zeros((B, NT, PG, 32, 3, WR), mybir.dt.np(DT8))
    ftp[:, :, :, :NF] = ftq.transpose(0, 1, 3, 4, 2, 5)
    ftp = ftp.reshape(B, NT, 128, 3 * WR)
    # xwd[b, t, 32*pix+(di*5+dj), (gsub, w, c)] = xp[b, c, 4*(3t+gsub)+pix+di, w+dj]
    xp = np.zeros((B, C, H + 2 * PAD, W + 2 * PAD), np.float16)
    xp[:, :, PAD : PAD + H, PAD : PAD + W] = x.astype(np.float16)
    xw = np.empty((B, NG, PG, K, K, W, C), np.float16)
    rows0 = np.arange(NG) * PG
    for pix in range(PG):
        for di in range(K):
            rows = rows0 + pix + di
            for dj in range(K):
                xw[:, :, pix, di, dj, :, :] = xp[:, :, rows, dj : dj + W].transpose(
                    0, 2, 3, 1
                )
    # [B, NG, PG, 25, WC] -> pad taps to 32 -> triples with gsub mid
    xwq = xw.reshape(B, NT, 3, PG, NF, WC)
    xwp = np.zeros((B, NT, PG, 32, 3, WC), np.float16)
    xwp[:, :, :, :NF] = xwq.transpose(0, 1, 3, 4, 2, 5)
    xwp = xwp.reshape(B, NT, 128, 3 * WC)[:, :, :122]
    xwp = np.ascontiguousarray(xwp)
    maps = []
    for b in range(B):
        maps.append({"ftd": ftp[b], "xwd": xwp[b]})
    return maps


def _decode_idx():
    """Index arrays mapping od[t, j, m, (gsub, col)] -> out[c*16+r, h, w]."""
    if "idx" in _CACHED:
        return _CACHED["idx"]
    cr = np.arange(C * R)[:, None, None]
    h = np.arange(H)[None, :, None]
    w = np.arange(W)[None, None, :]
    c, r = cr // R, cr % R
    g = h // PG
    pix = h % PG
    m = 3 * pix + c
    lo = w < 256
    j_lo = (w % 128) // 32
    j_hi = np.clip(w - 256, 0, None) // 16
    j = np.where(lo, j_lo, j_hi)
    col_lo = 512 * (w // 128) + 16 * (w % 32) + r
    col_hi = 1024 + 16 * (np.clip(w - 256, 0, None) % 16) + r
    col = np.where(lo, col_lo, col_hi)
    t = g // 3
    fullcol = (g % 3) * 1280 + col
    t_b = np.broadcast_to(t, (C * R, H, W))
    j_b = np.broadcast_to(j, (C * R, H, W))
    m_b = np.broadcast_to(m, (C * R, H, W))
    col_b = np.broadcast_to(fullcol, (C * R, H, W))
    _CACHED["idx"] = (t_b, j_b, m_b, col_b)
    return _CACHED["idx"]


def _decode(od_all):
    """od_all: [B, NT, 4, 12, 3840] fp16 -> [B, 48, H, W] fp32."""
    t_b, j_b, m_b, col_b = _decode_idx()
    out = np.empty((od_all.shape[0], C * R, H, W), np.float32)
    for b in range(od_all.shape[0]):
        out[b] = od_all[b][t_b, j_b, m_b, col_b].astype(np.float32)
    return out


def kernel(x: np.ndarray, filters: np.ndarray):
    nc = _get_nc()
    maps = _prep_maps(x, filters)
    res = run_bass_kernel_spmd(nc, maps, list(range(B)))
    od_all = np.stack([np.asarray(res.results[b]["od"]) for b in range(B)], axis=0)
    return _decode(od_all)


# revision 26
# speedup vs baseline: 1.0114x; 1.0114x over previous
"""DynamicUpsamplingFilter kernel for Trainium2 (Bass/Tile), 8 NeuronCores.

out[b, c*16+r, h, w] = sum_{di,dj} x_pad[b, c, h+di, w+dj] * filters[b, di*5+dj, r, h, w]

Sharding: purely data parallel - one batch element per NeuronCore (B=8).

Per-core dataflow (PE-centric; the per-pixel [3x25]@[25x16] contraction runs
directly on the tensor engine):
  * Image rows are grouped in chunks of PG=4 rows (NG=45 per core). Partition
    p = 32*pix + f holds tap f (of 25) for row-in-group pix (of 4); partitions
    32*pix+25..31 are dead (host ships zeros there; the matching stationary
    weight rows also stay zero, so those lanes contribute nothing).
  * Host prepacks 3 groups per DMA ("triples"). Filters go as fp8 E3M4 -
    filter values are uniform [0,1) so 4 mantissa bits keep the output L2
    error at ~1.24e-2, under the 2e-2 gate - and x windows go as fp16:
      ftd[t, p, (gsub, w, r)] = filters[f, r, 4(3t+gsub)+pix, w],  p = 32*pix+f
      xwd[t, p, (gsub, w, c)] = x_pad[c, 4(3t+gsub)+pix+di-2, w+dj-2]
  * DVE scatters xwd into per-group block-diagonal stationary weights
      w5b[32*pix+f, w, 3*pix+c] = xwd[...]   (all other slots stay zero)
    so for every (group, w) the [128, 16] stationary W holds the 4 pixels'
    25-tap patches on its block diagonal (cols 12..15 zero).
  * PE: ONE matmul per (group, w): out[16, 16] = W.T @ ftd[:, w, :] computes
    all 48 outputs (3 channels x 16 r) for 4 pixels at column w in a single
    instruction; fp8 moving operand, fp16 stationary, fp32 psum accumulate.
    Outputs land in psum col-strips via tile_position (0, 32j).
  * ACT drains psum -> SBUF fp16 into a per-triple staging tile; 4 strip
    stores per triple (partition base 32j, 12 rows each) write only the
    useful rows. The host reassembles the fp32 output (pure layout work,
    no arithmetic).
Measured: TimelineSim ~139.8 us per core (baseline was 413.6 us); verified on
8x TRN2 NeuronCores with L2 rel err 1.24e-2 vs the fp32 reference.
"""

import numpy as np

import concourse.bass as bass
import concourse.bacc as bacc
import concourse.mybir as mybir
from concourse.tile import TileContext
from concourse.bass_utils import run_bass_kernel_spmd

B, C, H, W = 8, 3, 180, 320
NF, R = 25, 16
K, PAD = 5, 2
PG = 4  # rows per group
NG = H // PG  # 45 groups
NT = NG // 3  # 15 triples
WR = W * R
WC = W * C
NFT = 2  # ft triple buffers
NXW = 2  # xw triple buffers
NST = 3  # store staging buffers
NW5 = 3  # block-diag weight buffers

DT = mybir.dt.float16
DT8 = mybir.dt.float8e3
F32 = mybir.dt.float32
I32 = mybir.dt.int32

_CACHED = {}


def _build_nc():
    nc = bacc.Bacc("TRN2", target_bir_lowering=False, debug=False, num_devices=8)
    ftd = nc.dram_tensor("ftd", [NT, 121, 3 * WR], DT8, kind="ExternalInput")
    ftd0 = nc.dram_tensor("ftd0", [NFT, 128, 3 * WR], DT8, kind="ExternalInput")
    xwd = nc.dram_tensor("xwd", [NT, 122, 3 * WC], DT, kind="ExternalInput")
    od = nc.dram_tensor("od", [NT, 4, 12, 3 * 1280], DT, kind="ExternalOutput")

    with TileContext(nc) as tc:
        with (
            tc.tile_pool(name="p", bufs=1) as pool,
            tc.tile_pool(name="ps", bufs=1, space="PSUM") as psp,
        ):
            w5bufs = [
                pool.tile([128, W, 16], DT, tag=f"w5{i}", name=f"w5{i}")
                for i in range(NW5)
            ]
            engs = [nc.vector, nc.gpsimd]
            for i, t in enumerate(w5bufs):
                engs[i % 2].memset(t[:].bitcast(I32), 0)
            ft_tiles = [
                pool.tile([128, 3, W, R], DT8, tag=f"ft{i}", name=f"ftt{i}")
                for i in range(NFT)
            ]
            xw_tiles = [
                pool.tile([128, 3, W, C], DT, tag=f"xw{i}", name=f"xwt{i}")
                for i in range(NXW)
            ]
            st_tiles = [
                pool.tile([128, 3, 1280], DT, tag=f"st{i}", name=f"stt{i}")
                for i in range(NST)
            ]
            for g in range(NG):
                t, gsub = g // 3, g % 3
                w5b = w5bufs[g % NW5]
                ftt = ft_tiles[t % NFT]
                xwt = xw_tiles[t % NXW]
                st = st_tiles[t % NST]
                if gsub == 0:
                    nc.sync.dma_start(
                        out=bass.AP(
                            xwt[:].tensor, 0, [[3 * WC, 122], [1, 3 * WC]]
                        ),
                        in_=xwd[t],
                    )
                    if t < NFT:
                        # first fill of each buffer ships all 128 rows so the
                        # trailing pad block (121..127) is zeroed once
                        nc.sync.dma_start(
                            out=bass.AP(
                                ftt[:].tensor, 0, [[3 * WR, 128], [1, 3 * WR]]
                            ),
                            in_=ftd0[t],
                        )
                    else:
                        nc.sync.dma_start(
                            out=bass.AP(
                                ftt[:].tensor, 0, [[3 * WR, 121], [1, 3 * WR]]
                            ),
                            in_=ftd[t],
                        )
                # scatter the 4-pixel patch blocks into the block-diag weights
                for pix in range(PG):
                    nc.vector.tensor_copy(
                        out=w5b[32 * pix : 32 * pix + NF, :, 3 * pix : 3 * pix + 3],
                        in_=xwt[32 * pix : 32 * pix + NF, gsub],
                    )
                pa = psp.tile([128, 1024], F32, tag="pa", bufs=2, name="pa")
                pc = psp.tile([128, 512], F32, tag="pc", bufs=2, name="pc")
                for w in range(W):
                    if w < 256:
                        j, blk, s = (w % 128) // 32, w // 128, w % 32
                        out = pa[
                            32 * j : 32 * j + 16,
                            512 * blk + 16 * s : 512 * blk + 16 * s + 16,
                        ]
                    else:
                        j, s = (w - 256) // 16, (w - 256) % 16
                        out = pc[32 * j : 32 * j + 16, 16 * s : 16 * s + 16]
                    nc.tensor.matmul(
                        out,
                        w5b[:, w, :],
                        ftt[:, gsub, w, :],
                        start=True,
                        stop=True,
                        tile_position=(0, 32 * j),
                    )
                nc.scalar.copy(out=st[:, gsub, :1024], in_=pa)
                nc.scalar.copy(out=st[:, gsub, 1024:1280], in_=pc[:, :256])
                if gsub == 2:
                    for j in range(4):
                        # last triple: alternate issue queues so the final
                        # stores' issue latency overlaps
                        eng = nc.sync if (t == NT - 1 and j % 2) else nc.scalar
                        eng.dma_start(
                            out=od[t, j],
                            in_=st[32 * j : 32 * j + 12],
                        )

    nc.compile()
    return nc


def _get_nc():
    if "nc" not in _CACHED:
        _CACHED["nc"] = _build_nc()
    return _CACHED["nc"]


def _prep_maps(x, filters):
    x = np.asarray(x)
    filters = np.asarray(filters)
    # ftd[b, t, 32*pix+f, (gsub, w, r)] = filters[b, f, r, 4*(3t+gsub)+pix, w]
    ftq = (
        filters.astype(mybir.dt.np(DT8))
        .transpose(0, 3, 1, 4, 2)  # [B, H, 25, W, 16]
        .reshape(B, NT, 3, PG, NF, WR)
    )
    ftp = np.zeros((B, NT, PG, 32, 3, WR), mybir.dt.np(DT8))
    ftp[:, :, :, :NF] = ftq.transpose(0, 1, 3, 4, 2, 5)
    ftp = ftp.reshape(B, NT, 128, 3 * WR)
    ftp0 = np.ascontiguousarray(ftp[:, :NFT])
    ftp = np.ascontiguousarray(ftp[:, :, :121])
    # xwd[b, t, 32*pix+(di*5+dj), (gsub, w, c)] = xp[b, c, 4*(3t+gsub)+pix+di, w+dj]
    xp = np.zeros((B, C, H + 2 * PAD, W + 2 * PAD), np.float16)
    xp[:, :, PAD : PAD + H, PAD : PAD + W] = x.astype(np.float16)
    xw = np.empty((B, NG, PG, K, K, W, C), np.float16)
    rows0 = np.arange(NG) * PG
    for pix in range(PG):
        for di in range(K):
            rows = rows0 + pix + di
            for dj in range(K):
                xw[:, :, pix, di, dj, :, :] = xp[:, :, rows, dj : dj + W].transpose(
                    0, 2, 3, 1
                )
    # [B, NG, PG, 25, WC] -> pad taps to 32 -> triples with gsub mid
    xwq = xw.reshape(B, NT, 3, PG, NF, WC)
    xwp = np.zeros((B, NT, PG, 32, 3, WC), np.float16)
    xwp[:, :, :, :NF] = xwq.transpose(0, 1, 3, 4, 2, 5)
    xwp = xwp.reshape(B, NT, 128, 3 * WC)[:, :, :122]
    xwp = np.ascontiguousarray(xwp)
    maps = []
    for b in range(B):
        maps.append({"ftd": ftp[b], "xwd": xwp[b], "ftd0": ftp0[b]})
    return maps


def _decode_idx():
    """Index arrays mapping od[t, j, m, (gsub, col)] -> out[c*16+r, h, w]."""
    if "idx" in _CACHED:
        return _CACHED["idx"]
    cr = np.arange(C * R)[:, None, None]
    h = np.arange(H)[None, :, None]
    w = np.arange(W)[None, None, :]
    c, r = cr // R, cr % R
    g = h // PG
    pix = h % PG
    m = 3 * pix + c
    lo = w < 256
    j_lo = (w % 128) // 32
    j_hi = np.clip(w - 256, 0, None) // 16
    j = np.where(lo, j_lo, j_hi)
    col_lo = 512 * (w // 128) + 16 * (w % 32) + r
    col_hi = 1024 + 16 * (np.clip(w - 256, 0, None) % 16) + r
    col = np.where(lo, col_lo, col_hi)
    t = g // 3
    fullcol = (g % 3) * 1280 + col
    t_b = np.broadcast_to(t, (C * R, H, W))
    j_b = np.broadcast_to(j, (C * R, H, W))
    m_b = np.broadcast_to(m, (C * R, H, W))
    col_b = np.broadcast_to(fullcol, (C * R, H, W))
    _CACHED["idx"] = (t_b, j_b, m_b, col_b)
    return _CACHED["idx"]


def _decode(od_all):
    """od_all: [B, NT, 4, 12, 3840] fp16 -> [B, 48, H, W] fp32."""
    t_b, j_b, m_b, col_b = _decode_idx()
    out = np.empty((od_all.shape[0], C * R, H, W), np.float32)
    for b in range(od_all.shape[0]):
        out[b] = od_all[b][t_b, j_b, m_b, col_b].astype(np.float32)
    return out


def kernel(x: np.ndarray, filters: np.ndarray):
    nc = _get_nc()
    maps = _prep_maps(x, filters)
    res = run_bass_kernel_spmd(nc, maps, list(range(B)))
    od_all = np.stack([np.asarray(res.results[b]["od"]) for b in range(B)], axis=0)
    return _decode(od_all)


# revision 28
# speedup vs baseline: 1.0135x; 1.0021x over previous
"""DynamicUpsamplingFilter kernel for Trainium2 (Bass/Tile), 8 NeuronCores.

out[b, c*16+r, h, w] = sum_{di,dj} x_pad[b, c, h+di, w+dj] * filters[b, di*5+dj, r, h, w]

Sharding: purely data parallel - one batch element per NeuronCore (B=8).

Per-core dataflow (PE-centric; the per-pixel [3x25]@[25x16] contraction runs
directly on the tensor engine):
  * Image rows are grouped in chunks of PG=4 rows (NG=45 per core). Partition
    p = 32*pix + f holds tap f (of 25) for row-in-group pix (of 4); partitions
    32*pix+25..31 are dead (host ships zeros there; the matching stationary
    weight rows also stay zero, so those lanes contribute nothing).
  * Host prepacks 3 groups per DMA ("triples"). Filters go as fp8 E3M4 -
    filter values are uniform [0,1) so 4 mantissa bits keep the output L2
    error at ~1.24e-2, under the 2e-2 gate - and x windows go as fp16:
      ftd[t, p, (gsub, w, r)] = filters[f, r, 4(3t+gsub)+pix, w],  p = 32*pix+f
      xwd[t, p, (gsub, w, c)] = x_pad[c, 4(3t+gsub)+pix+di-2, w+dj-2]
  * DVE scatters xwd into per-group block-diagonal stationary weights
      w5b[32*pix+f, w, 3*pix+c] = xwd[...]   (all other slots stay zero)
    so for every (group, w) the [128, 16] stationary W holds the 4 pixels'
    25-tap patches on its block diagonal (cols 12..15 zero).
  * PE: ONE matmul per (group, w): out[16, 16] = W.T @ ftd[:, w, :] computes
    all 48 outputs (3 channels x 16 r) for 4 pixels at column w in a single
    instruction; fp8 moving operand, fp16 stationary, fp32 psum accumulate.
    Outputs land in psum col-strips via tile_position (0, 32j).
  * ACT drains psum -> SBUF fp16 into a per-triple staging tile; 4 strip
    stores per triple (partition base 32j, 12 rows each) write only the
    useful rows. The host reassembles the fp32 output (pure layout work,
    no arithmetic).
Measured: TimelineSim 138.1 us per core (baseline was 413.6 us); verified on
8x TRN2 NeuronCores with L2 rel err 1.24e-2 vs the fp32 reference.
"""

import numpy as np

import concourse.bass as bass
import concourse.bacc as bacc
import concourse.mybir as mybir
from concourse.tile import TileContext
from concourse.bass_utils import run_bass_kernel_spmd

B, C, H, W = 8, 3, 180, 320
NF, R = 25, 16
K, PAD = 5, 2
PG = 4  # rows per group
NG = H // PG  # 45 groups
NT = NG // 3  # 15 triples
WR = W * R
WC = W * C
NFT = 2  # ft triple buffers
NXW = 3  # xw triple buffers
NST = 3  # store staging buffers
NW5 = 3  # block-diag weight buffers

DT = mybir.dt.float16
DT8 = mybir.dt.float8e3
F32 = mybir.dt.float32
I32 = mybir.dt.int32

_CACHED = {}


def _build_nc():
    nc = bacc.Bacc("TRN2", target_bir_lowering=False, debug=False, num_devices=8)
    ftd = nc.dram_tensor("ftd", [NT, 121, 3 * WR], DT8, kind="ExternalInput")
    ftd0 = nc.dram_tensor("ftd0", [NFT, 128, 3 * WR], DT8, kind="ExternalInput")
    xwd = nc.dram_tensor("xwd", [NT, 121, 3 * WC], DT, kind="ExternalInput")
    od = nc.dram_tensor("od", [NT, 4, 12, 3 * 1280], DT, kind="ExternalOutput")

    with TileContext(nc) as tc:
        with (
            tc.tile_pool(name="p", bufs=1) as pool,
            tc.tile_pool(name="ps", bufs=1, space="PSUM") as psp,
        ):
            w5bufs = [
                pool.tile([128, W, 16], DT, tag=f"w5{i}", name=f"w5{i}")
                for i in range(NW5)
            ]
            engs = [nc.vector, nc.gpsimd]
            for i, t in enumerate(w5bufs):
                engs[i % 2].memset(t[:].bitcast(I32), 0)
            ft_tiles = [
                pool.tile([128, 3, W, R], DT8, tag=f"ft{i}", name=f"ftt{i}")
                for i in range(NFT)
            ]
            xw_tiles = [
                pool.tile([128, 3, W, C], DT, tag=f"xw{i}", name=f"xwt{i}")
                for i in range(NXW)
            ]
            st_tiles = [
                pool.tile([128, 3, 1280], DT, tag=f"st{i}", name=f"stt{i}")
                for i in range(NST)
            ]
            for g in range(NG):
                t, gsub = g // 3, g % 3
                w5b = w5bufs[g % NW5]
                ftt = ft_tiles[t % NFT]
                xwt = xw_tiles[t % NXW]
                st = st_tiles[t % NST]
                if gsub == 0:
                    nc.sync.dma_start(
                        out=bass.AP(
                            xwt[:].tensor, 0, [[3 * WC, 121], [1, 3 * WC]]
                        ),
                        in_=xwd[t],
                    )
                    if t < NFT:
                        # first fill of each buffer ships all 128 rows so the
                        # trailing pad block (121..127) is zeroed once
                        nc.sync.dma_start(
                            out=bass.AP(
                                ftt[:].tensor, 0, [[3 * WR, 128], [1, 3 * WR]]
                            ),
                            in_=ftd0[t],
                        )
                    else:
                        nc.sync.dma_start(
                            out=bass.AP(
                                ftt[:].tensor, 0, [[3 * WR, 121], [1, 3 * WR]]
                            ),
                            in_=ftd[t],
                        )
                # scatter the 4-pixel patch blocks into the block-diag weights
                for pix in range(PG):
                    nc.vector.tensor_copy(
                        out=w5b[32 * pix : 32 * pix + NF, :, 3 * pix : 3 * pix + 3],
                        in_=xwt[32 * pix : 32 * pix + NF, gsub],
                    )
                pa = psp.tile([128, 1024], F32, tag="pa", bufs=2, name="pa")
                pc = psp.tile([128, 512], F32, tag="pc", bufs=2, name="pc")
                for w in range(W):
                    if w < 256:
                        j, blk, s = (w % 128) // 32, w // 128, w % 32
                        out = pa[
                            32 * j : 32 * j + 16,
                            512 * blk + 16 * s : 512 * blk + 16 * s + 16,
                        ]
                    else:
                        j, s = (w - 256) // 16, (w - 256) % 16
                        out = pc[32 * j : 32 * j + 16, 16 * s : 16 * s + 16]
                    nc.tensor.matmul(
                        out,
                        w5b[:, w, :],
                        ftt[:, gsub, w, :],
                        start=True,
                        stop=True,
                        tile_position=(0, 32 * j),
                    )
                nc.scalar.copy(out=st[:, gsub, :1024], in_=pa)
                nc.scalar.copy(out=st[:, gsub, 1024:1280], in_=pc[:, :256])
                if gsub == 2:
                    for j in range(4):
                        # last triple: alternate issue queues so the final
                        # stores' issue latency overlaps
                        eng = nc.sync if (t == NT - 1 and j % 2) else nc.scalar
                        eng.dma_start(
                            out=od[t, j],
                            in_=st[32 * j : 32 * j + 12],
                        )

    nc.compile()
    return nc


def _get_nc():
    if "nc" not in _CACHED:
        _CACHED["nc"] = _build_nc()
    return _CACHED["nc"]


def _prep_maps(x, filters):
    x = np.asarray(x)
    filters = np.asarray(filters)
    # ftd[b, t, 32*pix+f, (gsub, w, r)] = filters[b, f, r, 4*(3t+gsub)+pix, w]
    ftq = (
        filters.astype(mybir.dt.np(DT8))
        .transpose(0, 3, 1, 4, 2)  # [B, H, 25, W, 16]
        .reshape(B, NT, 3, PG, NF, WR)
    )
    ftp = np.zeros((B, NT, PG, 32, 3, WR), mybir.dt.np(DT8))
    ftp[:, :, :, :NF] = ftq.transpose(0, 1, 3, 4, 2, 5)
    ftp = ftp.reshape(B, NT, 128, 3 * WR)
    ftp0 = np.ascontiguousarray(ftp[:, :NFT])
    ftp = np.ascontiguousarray(ftp[:, :, :121])
    # xwd[b, t, 32*pix+(di*5+dj), (gsub, w, c)] = xp[b, c, 4*(3t+gsub)+pix+di, w+dj]
    xp = np.zeros((B, C, H + 2 * PAD, W + 2 * PAD), np.float16)
    xp[:, :, PAD : PAD + H, PAD : PAD + W] = x.astype(np.float16)
    xw = np.empty((B, NG, PG, K, K, W, C), np.float16)
    rows0 = np.arange(NG) * PG
    for pix in range(PG):
        for di in range(K):
            rows = rows0 + pix + di
            for dj in range(K):
                xw[:, :, pix, di, dj, :, :] = xp[:, :, rows, dj : dj + W].transpose(
                    0, 2, 3, 1
                )
    # [B, NG, PG, 25, WC] -> pad taps to 32 -> triples with gsub mid
    xwq = xw.reshape(B, NT, 3, PG, NF, WC)
    xwp = np.zeros((B, NT, PG, 32, 3, WC), np.float16)
    xwp[:, :, :, :NF] = xwq.transpose(0, 1, 3, 4, 2, 5)
    xwp = xwp.reshape(B, NT, 128, 3 * WC)[:, :, :121]
    xwp = np.ascontiguousarray(xwp)
    maps = []
    for b in range(B):
        maps.append({"ftd": ftp[b], "xwd": xwp[b], "ftd0": ftp0[b]})
    return maps


def _decode_idx():
    """Index arrays mapping od[t, j, m, (gsub, col)] -> out[c*16+r, h, w]."""
    if "idx" in _CACHED:
        return _CACHED["idx"]
    cr = np.arange(C * R)[:, None, None]
    h = np.arange(H)[None, :, None]
    w = np.arange(W)[None, None, :]
    c, r = cr // R, cr % R
    g = h // PG
    pix = h % PG
    m = 3 * pix + c
    lo = w < 256
    j_lo = (w % 128) // 32
    j_hi = np.clip(w - 256, 0, None) // 16
    j = np.where(lo, j_lo, j_hi)
    col_lo = 512 * (w // 128) + 16 * (w % 32) + r
    col_hi = 1024 + 16 * (np.clip(w - 256, 0, None) % 16) + r
    col = np.where(lo, col_lo, col_hi)
    t = g // 3
    fullcol = (g % 3) * 1280 + col
    t_b = np.broadcast_to(t, (C * R, H, W))
    j_b = np.broadcast_to(j, (C * R, H, W))
    m_b = np.broadcast_to(m, (C * R, H, W))
    col_b = np.broadcast_to(fullcol, (C * R, H, W))
    _CACHED["idx"] = (t_b, j_b, m_b, col_b)
    return _CACHED["idx"]


def _decode(od_all):
    """od_all: [B, NT, 4, 12, 3840] fp16 -> [B, 48, H, W] fp32."""
    t_b, j_b, m_b, col_b = _decode_idx()
    out = np.empty((od_all.shape[0], C * R, H, W), np.float32)
    for b in range(od_all.shape[0]):
        out[b] = od_all[b][t_b, j_b, m_b, col_b].astype(np.float32)
    return out


def kernel(x: np.ndarray, filters: np.ndarray):
    nc = _get_nc()
    maps = _prep_maps(x, filters)
    res = run_bass_kernel_spmd(nc, maps, list(range(B)))
    od_all = np.stack([np.asarray(res.results[b]["od"]) for b in range(B)], axis=0)
    return _decode(od_all)


# revision 30
# speedup vs baseline: 1.0167x; 1.0031x over previous
"""DynamicUpsamplingFilter kernel for Trainium2 (Bass/Tile), 8 NeuronCores.

out[b, c*16+r, h, w] = sum_{di,dj} x_pad[b, c, h+di, w+dj] * filters[b, di*5+dj, r, h, w]

Sharding: purely data parallel - one batch element per NeuronCore (B=8).

Per-core dataflow (PE-centric; the per-pixel [3x25]@[25x16] contraction runs
directly on the tensor engine):
  * Image rows are grouped in chunks of PG=4 rows (NG=45 per core). Partition
    p = 32*pix + f holds tap f (of 25) for row-in-group pix (of 4); partitions
    32*pix+25..31 are dead (host ships zeros there; the matching stationary
    weight rows also stay zero, so those lanes contribute nothing).
  * Host prepacks 3 groups per DMA ("triples"). Filters go as fp8 E3M4 -
    filter values are uniform [0,1) so 4 mantissa bits keep the output L2
    error at ~1.24e-2, under the 2e-2 gate - and x windows go as fp16:
      ftd[t, p, (gsub, w, r)] = filters[f, r, 4(3t+gsub)+pix, w],  p = 32*pix+f
      xwd[t, p, (gsub, w, c)] = x_pad[c, 4(3t+gsub)+pix+di-2, w+dj-2]
  * DVE scatters xwd into per-group block-diagonal stationary weights
      w5b[32*pix+f, w, 3*pix+c] = xwd[...]   (all other slots stay zero)
    so for every (group, w) the [128, 16] stationary W holds the 4 pixels'
    25-tap patches on its block diagonal (cols 12..15 zero).
  * PE: ONE matmul per (group, w): out[16, 16] = W.T @ ftd[:, w, :] computes
    all 48 outputs (3 channels x 16 r) for 4 pixels at column w in a single
    instruction; fp8 moving operand, fp16 stationary, fp32 psum accumulate.
    Outputs land in psum col-strips via tile_position (0, 32j).
  * ACT drains psum -> SBUF fp16 into a per-triple staging tile; 4 strip
    stores per triple (partition base 32j, 12 rows each) write only the
    useful rows. The host reassembles the fp32 output (pure layout work,
    no arithmetic).
Measured: TimelineSim 137.8 us per core (baseline was 413.6 us); verified on
8x TRN2 NeuronCores with L2 rel err 1.24e-2 vs the fp32 reference.
"""

import numpy as np

import concourse.bass as bass
import concourse.bacc as bacc
import concourse.mybir as mybir
from concourse.tile import TileContext
from concourse.bass_utils import run_bass_kernel_spmd

B, C, H, W = 8, 3, 180, 320
NF, R = 25, 16
K, PAD = 5, 2
PG = 4  # rows per group
NG = H // PG  # 45 groups
NT = NG // 3  # 15 triples
WR = W * R
WC = W * C
NFT = 2  # ft triple buffers
NXW = 3  # xw triple buffers
NST = 4  # store staging buffers
NW5 = 3  # block-diag weight buffers

DT = mybir.dt.float16
DT8 = mybir.dt.float8e3
F32 = mybir.dt.float32
I32 = mybir.dt.int32

_CACHED = {}


def _build_nc():
    nc = bacc.Bacc("TRN2", target_bir_lowering=False, debug=False, num_devices=8)
    ftd = nc.dram_tensor("ftd", [NT, 121, 3 * WR], DT8, kind="ExternalInput")
    ftd0 = nc.dram_tensor("ftd0", [NFT, 128, 3 * WR], DT8, kind="ExternalInput")
    xwd = nc.dram_tensor("xwd", [NT, 121, 3 * WC], DT, kind="ExternalInput")
    od = nc.dram_tensor("od", [NT, 4, 12, 3 * 1280], DT, kind="ExternalOutput")

    with TileContext(nc) as tc:
        with (
            tc.tile_pool(name="p", bufs=1) as pool,
            tc.tile_pool(name="ps", bufs=1, space="PSUM") as psp,
        ):
            w5bufs = [
                pool.tile([128, W, 16], DT, tag=f"w5{i}", name=f"w5{i}")
                for i in range(NW5)
            ]
            engs = [nc.vector, nc.gpsimd]
            for i, t in enumerate(w5bufs):
                engs[i % 2].memset(t[:].bitcast(I32), 0)
            ft_tiles = [
                pool.tile([128, 3, W, R], DT8, tag=f"ft{i}", name=f"ftt{i}")
                for i in range(NFT)
            ]
            xw_tiles = [
                pool.tile([128, 3, W, C], DT, tag=f"xw{i}", name=f"xwt{i}")
                for i in range(NXW)
            ]
            st_tiles = [
                pool.tile([128, 3, 1280], DT, tag=f"st{i}", name=f"stt{i}")
                for i in range(NST)
            ]
            for g in range(NG):
                t, gsub = g // 3, g % 3
                w5b = w5bufs[g % NW5]
                ftt = ft_tiles[t % NFT]
                xwt = xw_tiles[t % NXW]
                st = st_tiles[t % NST]
                if gsub == 0:
                    nc.sync.dma_start(
                        out=bass.AP(
                            xwt[:].tensor, 0, [[3 * WC, 121], [1, 3 * WC]]
                        ),
                        in_=xwd[t],
                    )
                    if t < NFT:
                        # first fill of each buffer ships all 128 rows so the
                        # trailing pad block (121..127) is zeroed once
                        nc.sync.dma_start(
                            out=bass.AP(
                                ftt[:].tensor, 0, [[3 * WR, 128], [1, 3 * WR]]
                            ),
                            in_=ftd0[t],
                        )
                    else:
                        nc.sync.dma_start(
                            out=bass.AP(
                                ftt[:].tensor, 0, [[3 * WR, 121], [1, 3 * WR]]
                            ),
                            in_=ftd[t],
                        )
                # scatter the 4-pixel patch blocks into the block-diag weights
                for pix in range(PG):
                    nc.vector.tensor_copy(
                        out=w5b[32 * pix : 32 * pix + NF, :, 3 * pix : 3 * pix + 3],
                        in_=xwt[32 * pix : 32 * pix + NF, gsub],
                    )
                pa = psp.tile([128, 1024], F32, tag="pa", bufs=2, name="pa")
                pc = psp.tile([128, 512], F32, tag="pc", bufs=2, name="pc")
                for w in range(W):
                    if w < 256:
                        j, blk, s = (w % 128) // 32, w // 128, w % 32
                        out = pa[
                            32 * j : 32 * j + 16,
                            512 * blk + 16 * s : 512 * blk + 16 * s + 16,
                        ]
                    else:
                        j, s = (w - 256) // 16, (w - 256) % 16
                        out = pc[32 * j : 32 * j + 16, 16 * s : 16 * s + 16]
                    nc.tensor.matmul(
                        out,
                        w5b[:, w, :],
                        ftt[:, gsub, w, :],
                        start=True,
                        stop=True,
                        tile_position=(0, 32 * j),
                    )
                nc.scalar.copy(out=st[:, gsub, :1024], in_=pa)
                nc.scalar.copy(out=st[:, gsub, 1024:1280], in_=pc[:, :256])
                if gsub == 2:
                    for j in range(4):
                        # last triple: alternate issue queues so the final
                        # stores' issue latency overlaps
                        eng = nc.sync if (t == NT - 1 and j % 2) else nc.scalar
                        eng.dma_start(
                            out=od[t, j],
                            in_=st[32 * j : 32 * j + 12],
                        )

    nc.compile()
    return nc


def _get_nc():
    if "nc" not in _CACHED:
        _CACHED["nc"] = _build_nc()
    return _CACHED["nc"]


def _prep_maps(x, filters):
    x = np.asarray(x)
    filters = np.asarray(filters)
    # ftd[b, t, 32*pix+f, (gsub, w, r)] = filters[b, f, r, 4*(3t+gsub)+pix, w]
    ftq = (
        filters.astype(mybir.dt.np(DT8))
        .transpose(0, 3, 1, 4, 2)  # [B, H, 25, W, 16]
        .reshape(B, NT, 3, PG, NF, WR)
    )
    ftp = np.zeros((B, NT, PG, 32, 3, WR), mybir.dt.np(DT8))
    ftp[:, :, :, :NF] = ftq.transpose(0, 1, 3, 4, 2, 5)
    ftp = ftp.reshape(B, NT, 128, 3 * WR)
    ftp0 = np.ascontiguousarray(ftp[:, :NFT])
    ftp = np.ascontiguousarray(ftp[:, :, :121])
    # xwd[b, t, 32*pix+(di*5+dj), (gsub, w, c)] = xp[b, c, 4*(3t+gsub)+pix+di, w+dj]
    xp = np.zeros((B, C, H + 2 * PAD, W + 2 * PAD), np.float16)
    xp[:, :, PAD : PAD + H, PAD : PAD + W] = x.astype(np.float16)
    xw = np.empty((B, NG, PG, K, K, W, C), np.float16)
    rows0 = np.arange(NG) * PG
    for pix in range(PG):
        for di in range(K):
            rows = rows0 + pix + di
            for dj in range(K):
                xw[:, :, pix, di, dj, :, :] = xp[:, :, rows, dj : dj + W].transpose(
                    0, 2, 3, 1
                )
    # [B, NG, PG, 25, WC] -> pad taps to 32 -> triples with gsub mid
    xwq = xw.reshape(B, NT, 3, PG, NF, WC)
    xwp = np.zeros((B, NT, PG, 32, 3, WC), np.float16)
    xwp[:, :, :, :NF] = xwq.transpose(0, 1, 3, 4, 2, 5)
    xwp = xwp.reshape(B, NT, 128, 3 * WC)[:, :, :121]
    xwp = np.ascontiguousarray(xwp)
    maps = []
    for b in range(B):
        maps.append({"ftd": ftp[b], "xwd": xwp[b], "ftd0": ftp0[b]})
    return maps


def _decode_idx():
    """Index arrays mapping od[t, j, m, (gsub, col)] -> out[c*16+r, h, w]."""
    if "idx" in _CACHED:
        return _CACHED["idx"]
    cr = np.arange(C * R)[:, None, None]
    h = np.arange(H)[None, :, None]
    w = np.arange(W)[None, None, :]
    c, r = cr // R, cr % R
    g = h // PG
    pix = h % PG
    m = 3 * pix + c
    lo = w < 256
    j_lo = (w % 128) // 32
    j_hi = np.clip(w - 256, 0, None) // 16
    j = np.where(lo, j_lo, j_hi)
    col_lo = 512 * (w // 128) + 16 * (w % 32) + r
    col_hi = 1024 + 16 * (np.clip(w - 256, 0, None) % 16) + r
    col = np.where(lo, col_lo, col_hi)
    t = g // 3
    fullcol = (g % 3) * 1280 + col
    t_b = np.broadcast_to(t, (C * R, H, W))
    j_b = np.broadcast_to(j, (C * R, H, W))
    m_b = np.broadcast_to(m, (C * R, H, W))
    col_b = np.broadcast_to(fullcol, (C * R, H, W))
    _CACHED["idx"] = (t_b, j_b, m_b, col_b)
    return _CACHED["idx"]


def _decode(od_all):
    """od_all: [B, NT, 4, 12, 3840] fp16 -> [B, 48, H, W] fp32."""
    t_b, j_b, m_b, col_b = _decode_idx()
    out = np.empty((od_all.shape[0], C * R, H, W), np.float32)
    for b in range(od_all.shape[0]):
        out[b] = od_all[b][t_b, j_b, m_b, col_b].astype(np.float32)
    return out


def kernel(x: np.ndarray, filters: np.ndarray):
    nc = _get_nc()
    maps = _prep_maps(x, filters)
    res = run_bass_kernel_spmd(nc, maps, list(range(B)))
    od_all = np.stack([np.asarray(res.results[b]["od"]) for b in range(B)], axis=0)
    return _decode(od_all)


# revision 32
# speedup vs baseline: 1.0219x; 1.0052x over previous
"""DynamicUpsamplingFilter kernel for Trainium2 (Bass/Tile), 8 NeuronCores.

out[b, c*16+r, h, w] = sum_{di,dj} x_pad[b, c, h+di, w+dj] * filters[b, di*5+dj, r, h, w]

Sharding: purely data parallel - one batch element per NeuronCore (B=8).

Per-core dataflow (PE-centric; the per-pixel [3x25]@[25x16] contraction runs
directly on the tensor engine):
  * Image rows are grouped in chunks of PG=4 rows (NG=45 per core). Partition
    p = 32*pix + f holds tap f (of 25) for row-in-group pix (of 4); partitions
    32*pix+25..31 are dead (host ships zeros there; the matching stationary
    weight rows also stay zero, so those lanes contribute nothing).
  * Host prepacks 3 groups per DMA ("triples"). Filters go as fp8 E3M4 -
    filter values are uniform [0,1) so 4 mantissa bits keep the output L2
    error at ~1.24e-2, under the 2e-2 gate - and x windows go as fp16:
      ftd[t, p, (gsub, w, r)] = filters[f, r, 4(3t+gsub)+pix, w],  p = 32*pix+f
      xwd[t, p, (gsub, w, c)] = x_pad[c, 4(3t+gsub)+pix+di-2, w+dj-2]
  * DVE scatters xwd into per-group block-diagonal stationary weights
      w5b[32*pix+f, w, 3*pix+c] = xwd[...]   (all other slots stay zero)
    so for every (group, w) the [128, 16] stationary W holds the 4 pixels'
    25-tap patches on its block diagonal (cols 12..15 zero).
  * PE: ONE matmul per (group, w): out[16, 16] = W.T @ ftd[:, w, :] computes
    all 48 outputs (3 channels x 16 r) for 4 pixels at column w in a single
    instruction; fp8 moving operand, fp16 stationary, fp32 psum accumulate.
    Outputs land in psum col-strips via tile_position (0, 32j).
  * ACT drains psum -> SBUF fp16 into a per-triple staging tile; 4 strip
    stores per triple (partition base 32j, 12 rows each) write only the
    useful rows. The host reassembles the fp32 output (pure layout work,
    no arithmetic).
Measured: TimelineSim 137.4 us per core (baseline was 413.6 us); verified on
8x TRN2 NeuronCores with L2 rel err 1.24e-2 vs the fp32 reference.
"""

import numpy as np

import concourse.bass as bass
import concourse.bacc as bacc
import concourse.mybir as mybir
from concourse.tile import TileContext
from concourse.bass_utils import run_bass_kernel_spmd

B, C, H, W = 8, 3, 180, 320
NF, R = 25, 16
K, PAD = 5, 2
PG = 4  # rows per group
NG = H // PG  # 45 groups
NT = NG // 3  # 15 triples
WR = W * R
WC = W * C
NFT = 2  # ft triple buffers
NXW = 3  # xw triple buffers
NST = 4  # store staging buffers
NW5 = 3  # block-diag weight buffers

DT = mybir.dt.float16
DT8 = mybir.dt.float8e3
F32 = mybir.dt.float32
I32 = mybir.dt.int32

_CACHED = {}


def _build_nc():
    nc = bacc.Bacc("TRN2", target_bir_lowering=False, debug=False, num_devices=8)
    ftd = nc.dram_tensor("ftd", [NT, 121, 3 * WR], DT8, kind="ExternalInput")
    ftd0 = nc.dram_tensor("ftd0", [NFT, 128, 3 * WR], DT8, kind="ExternalInput")
    xwd = nc.dram_tensor("xwd", [NT, 121, 3 * WC], DT, kind="ExternalInput")
    od = nc.dram_tensor("od", [NT, 4, 12, 3 * 1280], DT, kind="ExternalOutput")

    with TileContext(nc) as tc:
        with (
            tc.tile_pool(name="p", bufs=1) as pool,
            tc.tile_pool(name="ps", bufs=1, space="PSUM") as psp,
        ):
            w5bufs = [
                pool.tile([128, W, 16], DT, tag=f"w5{i}", name=f"w5{i}")
                for i in range(NW5)
            ]
            engs = [nc.vector, nc.gpsimd]
            for i, t in enumerate(w5bufs):
                engs[i % 2].memset(t[:].bitcast(I32), 0)
            ft_tiles = [
                pool.tile([128, 3, W, R], DT8, tag=f"ft{i}", name=f"ftt{i}")
                for i in range(NFT)
            ]
            xw_tiles = [
                pool.tile([128, 3, W, C], DT, tag=f"xw{i}", name=f"xwt{i}")
                for i in range(NXW)
            ]
            st_tiles = [
                pool.tile([128, 3, 1280], DT, tag=f"st{i}", name=f"stt{i}")
                for i in range(NST)
            ]
            for g in range(NG):
                t, gsub = g // 3, g % 3
                w5b = w5bufs[g % NW5]
                ftt = ft_tiles[t % NFT]
                xwt = xw_tiles[t % NXW]
                st = st_tiles[t % NST]
                if gsub == 0:
                    nc.sync.dma_start(
                        out=bass.AP(
                            xwt[:].tensor, 0, [[3 * WC, 121], [1, 3 * WC]]
                        ),
                        in_=xwd[t],
                    )
                    if t < NFT:
                        # first fill of each buffer ships all 128 rows so the
                        # trailing pad block (121..127) is zeroed once
                        nc.sync.dma_start(
                            out=bass.AP(
                                ftt[:].tensor, 0, [[3 * WR, 128], [1, 3 * WR]]
                            ),
                            in_=ftd0[t],
                        )
                    else:
                        nc.sync.dma_start(
                            out=bass.AP(
                                ftt[:].tensor, 0, [[3 * WR, 121], [1, 3 * WR]]
                            ),
                            in_=ftd[t],
                        )
                # scatter the 4-pixel patch blocks into the block-diag weights
                for pix in range(PG):
                    nc.vector.tensor_copy(
                        out=w5b[32 * pix : 32 * pix + NF, :, 3 * pix : 3 * pix + 3],
                        in_=xwt[32 * pix : 32 * pix + NF, gsub],
                    )
                pa = psp.tile([128, 1024], F32, tag="pa", bufs=2, name="pa")
                pc = psp.tile([128, 512], F32, tag="pc", bufs=3, name="pc")
                for w in range(W):
                    if w < 256:
                        j, blk, s = (w % 128) // 32, w // 128, w % 32
                        out = pa[
                            32 * j : 32 * j + 16,
                            512 * blk + 16 * s : 512 * blk + 16 * s + 16,
                        ]
                    else:
                        j, s = (w - 256) // 16, (w - 256) % 16
                        out = pc[32 * j : 32 * j + 16, 16 * s : 16 * s + 16]
                    nc.tensor.matmul(
                        out,
                        w5b[:, w, :],
                        ftt[:, gsub, w, :],
                        start=True,
                        stop=True,
                        tile_position=(0, 32 * j),
                    )
                nc.scalar.copy(out=st[:, gsub, :1024], in_=pa)
                nc.scalar.copy(out=st[:, gsub, 1024:1280], in_=pc[:, :256])
                if gsub == 2:
                    for j in range(4):
                        # last triple: alternate issue queues so the final
                        # stores' issue latency overlaps
                        eng = nc.sync if (t == NT - 1 and j % 2) else nc.scalar
                        eng.dma_start(
                            out=od[t, j],
                            in_=st[32 * j : 32 * j + 12],
                        )

    nc.compile()
    return nc


def _get_nc():
    if "nc" not in _CACHED:
        _CACHED["nc"] = _build_nc()
    return _CACHED["nc"]


def _prep_maps(x, filters):
    x = np.asarray(x)
    filters = np.asarray(filters)
    # ftd[b, t, 32*pix+f, (gsub, w, r)] = filters[b, f, r, 4*(3t+gsub)+pix, w]
    ftq = (
        filters.astype(mybir.dt.np(DT8))
        .transpose(0, 3, 1, 4, 2)  # [B, H, 25, W, 16]
        .reshape(B, NT, 3, PG, NF, WR)
    )
    ftp = np.zeros((B, NT, PG, 32, 3, WR), mybir.dt.np(DT8))
    ftp[:, :, :, :NF] = ftq.transpose(0, 1, 3, 4, 2, 5)
    ftp = ftp.reshape(B, NT, 128, 3 * WR)
    ftp0 = np.ascontiguousarray(ftp[:, :NFT])
    ftp = np.ascontiguousarray(ftp[:, :, :121])
    # xwd[b, t, 32*pix+(di*5+dj), (gsub, w, c)] = xp[b, c, 4*(3t+gsub)+pix+di, w+dj]
    xp = np.zeros((B, C, H + 2 * PAD, W + 2 * PAD), np.float16)
    xp[:, :, PAD : PAD + H, PAD : PAD + W] = x.astype(np.float16)
    xw = np.empty((B, NG, PG, K, K, W, C), np.float16)
    rows0 = np.arange(NG) * PG
    for pix in range(PG):
        for di in range(K):
            rows = rows0 + pix + di
            for dj in range(K):
                xw[:, :, pix, di, dj, :, :] = xp[:, :, rows, dj : dj + W].transpose(
                    0, 2, 3, 1
                )
    # [B, NG, PG, 25, WC] -> pad taps to 32 -> triples with gsub mid
    xwq = xw.reshape(B, NT, 3, PG, NF, WC)
    xwp = np.zeros((B, NT, PG, 32, 3, WC), np.float16)
    xwp[:, :, :, :NF] = xwq.transpose(0, 1, 3, 4, 2, 5)
    xwp = xwp.reshape(B, NT, 128, 3 * WC)[:, :, :121]
    xwp = np.ascontiguousarray(xwp)
    maps = []
    for b in range(B):
        maps.append({"ftd": ftp[b], "xwd": xwp[b], "ftd0": ftp0[b]})
    return maps


def _decode_idx():
    """Index arrays mapping od[t, j, m, (gsub, col)] -> out[c*16+r, h, w]."""
    if "idx" in _CACHED:
        return _CACHED["idx"]
    cr = np.arange(C * R)[:, None, None]
    h = np.arange(H)[None, :, None]
    w = np.arange(W)[None, None, :]
    c, r = cr // R, cr % R
    g = h // PG
    pix = h % PG
    m = 3 * pix + c
    lo = w < 256
    j_lo = (w % 128) // 32
    j_hi = np.clip(w - 256, 0, None) // 16
    j = np.where(lo, j_lo, j_hi)
    col_lo = 512 * (w // 128) + 16 * (w % 32) + r
    col_hi = 1024 + 16 * (np.clip(w - 256, 0, None) % 16) + r
    col = np.where(lo, col_lo, col_hi)
    t = g // 3
    fullcol = (g % 3) * 1280 + col
    t_b = np.broadcast_to(t, (C * R, H, W))
    j_b = np.broadcast_to(j, (C * R, H, W))
    m_b = np.broadcast_to(m, (C * R, H, W))
    col_b = np.broadcast_to(fullcol, (C * R, H, W))
    _CACHED["idx"] = (t_b, j_b, m_b, col_b)
    return _CACHED["idx"]


def _decode(od_all):
    """od_all: [B, NT, 4, 12, 3840] fp16 -> [B, 48, H, W] fp32."""
    t_b, j_b, m_b, col_b = _decode_idx()
    out = np.empty((od_all.shape[0], C * R, H, W), np.float32)
    for b in range(od_all.shape[0]):
        out[b] = od_all[b][t_b, j_b, m_b, col_b].astype(np.float32)
    return out


def kernel(x: np.ndarray, filters: np.ndarray):
    nc = _get_nc()
    maps = _prep_maps(x, filters)
    res = run_bass_kernel_spmd(nc, maps, list(range(B)))
    od_all = np.stack([np.asarray(res.results[b]["od"]) for b in range(B)], axis=0)
    return _decode(od_all)
